# revision 17
# baseline (speedup 1.0000x reference)
"""TRN2 Bass kernel for nn_Actor (retrieval_knn).

Data-parallel over batch across 8 NeuronCores (8192 rows/core).

Cost model measured on this axon-tunneled environment:
- statically-streamed instructions cost ~20-80us each (the engine
  queues are re-streamed per straight-line copy of the code), while
  For_i hardware-loop REPLAYS of the same instructions cost ~1-10us.
- ops with register-offset (bass.ds) operands are ~60-110us on
  DVE/Act/Pool engines but near-free on DMA queues and matmul rhs.
- mixing gpsimd libraries (ap_gather vs tensor ops) forces Q7 library
  reloads (~60us each).
Hence: the per-tile staging copy and argmax-output write use DMA / DVE
rather than DVE-with-ds copies, the bf16 split runs on DVE (not
gpsimd), and _build(L>1) wraps the body in a For_i hardware loop so
repeated execution replays cached instructions instead of re-streaming
them (the L=1 kernel is a single straight-line body).

Body structure (unchanged math):

- ONE ap_gather with 128 channels (all 8 Q7 cores busy): 4 groups of
  32 partitions, group g = batch slice g*2048:(g+1)*2048 with worker
  dims at rows +0:10 (one Q7 core, worker idx stream) and project dims
  at rows +16:26 (the next core, project idx stream) -> x64 [128, 2048].
- MLP layer 1 as 8 matmuls contracting all 128 partitions against two
  [128, 80] pair-block lhsTs (zero rows kill the other pair's groups),
  each computing two batch slices at once ([80, 512] PSUM outputs),
  2 fused ReLUs -> h80 [80, 4096]; the resulting batch-column
  permutation is undone on the host in _decode.
- h split into bf16 pair (h1, h2 = h - h1) with 2 elementwise ops, then
  6 DMAs assemble the 122-row stacked lhs (W2 absorbed into the table:
  scores = h1G1 + h1G2 + h2G1 + c1 + c2, fp32-grade accuracy).
- Scores + argmax: per 128-row tile, 5 matmuls (PSUM-bank-sized) with
  lhsT taken directly from the stacked buffer (no staging copy), then
  DVE max8 + max_index on the fp32 PSUM scores: exact argmax.
"""
import sys
sys.path.insert(0, '/opt/trn_rl_repo')
import numpy as np
import ml_dtypes

B = 65536
NCORES = 8
BC = B // NCORES            # 8192
NW, NPTAB, EMB = 1807, 2490, 10
NPROJ = NPTAB - 1           # 2489
HID = 40
NTILES = BC // 128          # 64
HALF = BC // 2              # 4096

_cache = {}


def _bf16(x):
    return np.asarray(x, np.float32).astype(ml_dtypes.bfloat16)


def _build(L=1, hwloop=True, tk=1, sr=True, cpe="dma", _si=None, _sg=False):
    from concourse import bacc, mybir, bass
    from concourse.tile import TileContext
    import concourse.mybir as mb
    dt = mybir.dt
    AF = mb.ActivationFunctionType
    nc = bacc.Bacc("TRN2", target_bir_lowering=False, debug=False, num_devices=NCORES)

    xidx = nc.dram_tensor("xidx", [128, BC // 4 // 16], dt.int16, kind="ExternalInput")
    xtab = nc.dram_tensor("xtab", [128, NPTAB], dt.float32, kind="ExternalInput")
    w1s = nc.dram_tensor("w1s", [128, 160], dt.float32, kind="ExternalInput")
    b1e = nc.dram_tensor("b1e", [80, 1], dt.float32, kind="ExternalInput")
    tstk = nc.dram_tensor("tstk", [122, NPROJ], dt.bfloat16, kind="ExternalInput")
    out_ext = nc.dram_tensor("out", [128, NTILES * 8], dt.uint32, kind="ExternalOutput")

    with TileContext(nc) as tc:
        with tc.tile_pool(name="const", bufs=1) as cp, \
             tc.tile_pool(name="work", bufs=1) as wp, \
             tc.tile_pool(name="sc", bufs=1, space="PSUM") as scp:
            t_xtab = cp.tile([128, NPTAB], dt.float32)
            t_xidx = cp.tile([128, BC // 4 // 16], dt.int16)
            t_w1s = cp.tile([128, 160], dt.float32)
            t_b1 = cp.tile([80, 1], dt.float32)
            t_tstk = cp.tile([122, NPROJ], dt.bfloat16)
            nc.sync.dma_start(out=t_xtab, in_=xtab.ap())
            nc.sync.dma_start(out=t_xidx, in_=xidx.ap())
            nc.sync.dma_start(out=t_w1s, in_=w1s.ap())
            nc.sync.dma_start(out=t_b1, in_=b1e.ap())
            nc.sync.dma_start(out=t_tstk, in_=tstk.ap())

            x64 = wp.tile([128, BC // 4], dt.float32)
            h80 = wp.tile([80, BC // 2], dt.float32)
            hs1 = wp.tile([80, BC // 2], dt.bfloat16)
            hs2 = wp.tile([80, BC // 2], dt.bfloat16)
            hstack = wp.tile([122, BC], dt.bfloat16)
            onesrow = wp.tile([2, BC], dt.bfloat16)
            outbuf = wp.tile([128, NTILES * 8], dt.uint32)
            wstage = wp.tile([122, 128 * tk], dt.bfloat16)
            m8 = wp.tile([128, 8], dt.float32)
            i8stage = wp.tile([128, 32], dt.uint32)
            nc.vector.memset(onesrow, 1.0)
            nc.sync.dma_start(out=hstack[120:122, :], in_=onesrow)
            ps = scp.tile([128, NPROJ], dt.float32)

            from concourse import library_config
            nc.gpsimd.load_library(library_config.ap_gather)
            from contextlib import nullcontext
            lctx = tc.For_i(0, L, 1, staggered_reset=True) if L > 1 \
                else nullcontext()
            with lctx:
                if _sg:
                    nc.vector.memset(x64[:, 0:8], 0.1)
                if not _sg:
                    nc.gpsimd.ap_gather(out_ap=x64, in_ap=t_xtab,
                                        idxs_ap=t_xidx,
                                        channels=128, num_elems=NPTAB, d=1,
                                        num_idxs=BC // 4)
                # MLP layer 1: [64,80] block lhsT -> [80,512] PSUM chunks,
                # two fused 4-bank ReLUs (fully unrolled: only 10 instructions)
                for grp in range(2):
                    for k in range(4):
                        nc.tensor.matmul(ps[0:80, k * 512:(k + 1) * 512],
                                         lhsT=t_w1s[:, grp * 80:(grp + 1) * 80],
                                         rhs=x64[:, k * 512:(k + 1) * 512],
                                         start=True, stop=True)
                    nc.scalar.activation(h80[:, grp * 2048:(grp + 1) * 2048],
                                         ps[0:80, 0:2048],
                                         mb.ActivationFunctionType.Relu,
                                         bias=t_b1)
                # bf16 split and 122-row lhs assembly
                nc.vector.tensor_copy(hs1[:, :], h80[:, :])
                nc.vector.tensor_sub(hs2[:, :], h80[:, :], hs1[:, :])
                nc.sync.dma_start(out=hstack[0:40, 0:HALF], in_=hs1[0:40, :])
                nc.scalar.dma_start(out=hstack[0:40, HALF:BC], in_=hs1[40:80, :])
                nc.sync.dma_start(out=hstack[64:104, 0:HALF], in_=hs2[0:40, :])
                nc.scalar.dma_start(out=hstack[64:104, HALF:BC], in_=hs2[40:80, :])
                nc.sync.dma_start(out=hstack[40:64, :], in_=hstack[0:24, :])
                nc.scalar.dma_start(out=hstack[104:120, :], in_=hstack[24:40, :])

                # scores + argmax: tk tiles per loop iteration share ONE
                # wstage group copy (ldweights needs static lhsT addresses,
                # which the static slices of the group buffer provide); each
                # tile is 5 bank-sized matmuls + exact max8/max_index
                def tile_group(iv):
                    src = hstack[:, bass.ds(iv * 128 * tk, 128 * tk)]
                    if cpe == "scalar":
                        nc.scalar.activation(wstage[:, :], src, AF.Copy)
                    elif cpe == "dma":
                        nc.sync.dma_start(out=wstage[:, :], in_=src)
                    else:
                        nc.vector.tensor_copy(wstage[:, :], src)
                    for r in range(tk):
                        lhs = wstage[:, r * 128:(r + 1) * 128]
                        for s0 in range(0, NPROJ, 512):
                            sw = min(512, NPROJ - s0)
                            nc.tensor.matmul(ps[:, s0:s0 + sw], lhsT=lhs,
                                             rhs=t_tstk[:, s0:s0 + sw],
                                             start=True, stop=True)
                        nc.vector.max(out=m8, in_=ps[:, 0:NPROJ])
                        # static-address maxidx output; a DMA does the
                        # register-offset scatter (dyn-offset APs are
                        # near-free on DMA queues but slow on DVE)
                        nc.vector.max_index(
                            out=i8stage[:, r * 8:(r + 1) * 8],
                            in_max=m8, in_values=ps[:, 0:NPROJ])
                    nc.scalar.dma_start(
                        out=outbuf[:, bass.ds(iv * 8 * tk, 8 * tk)],
                        in_=i8stage[:, 0:8 * tk])
                if hwloop:
                    with tc.For_i(0, _si or (NTILES // tk), 1, staggered_reset=sr) as iv:
                        tile_group(iv)
                else:
                    for k in range(NTILES // tk):
                        tile_group(k)

            nc.sync.dma_start(out=out_ext.ap(), in_=outbuf)
    nc.compile()
    return nc


def _host_prep(inputs):
    worker_ids = np.asarray(inputs["worker_ids"]).astype(np.int64)
    project_ids = np.asarray(inputs["project_ids"]).astype(np.int64)
    worker_emb = np.asarray(inputs["worker_emb"], dtype=np.float32)
    project_emb = np.asarray(inputs["project_emb"], dtype=np.float32)
    W1 = np.asarray(inputs["W1"], dtype=np.float32)
    b1 = np.asarray(inputs["b1"], dtype=np.float32)
    W2 = np.asarray(inputs["W2"], dtype=np.float32)
    b2 = np.asarray(inputs["b2"], dtype=np.float32)

    table = project_emb[1:]
    G = (table @ W2).astype(np.float32)
    c = (table @ b2).astype(np.float32)
    G1 = _bf16(G)
    G2 = _bf16(G - G1.astype(np.float32))
    c1 = _bf16(c)
    c2 = _bf16(c - c1.astype(np.float32))
    tstk = np.zeros((122, NPROJ), dtype=ml_dtypes.bfloat16)
    tstk[0:40] = G1.T
    tstk[40:64] = G2.T[0:24]
    tstk[64:104] = G1.T
    tstk[104:120] = G2.T[24:40]
    tstk[120] = c1
    tstk[121] = c2

    # combined gather table: 4 groups of 32 partitions (all 8 Q7 cores);
    # group g = batch slice g*2048:(g+1)*2048: rows +0:10 worker dims
    # (core 2g, worker idx stream), rows +16:26 project dims (core 2g+1)
    xtab = np.zeros((128, NPTAB), dtype=np.float32)
    for g in range(4):
        xtab[32 * g:32 * g + EMB, 0:NW] = worker_emb.T
        xtab[32 * g + 16:32 * g + 16 + EMB] = project_emb.T

    # pair-block lhsT [128, 160]: col block p*80 serves groups (2p, 2p+1):
    # group 2p -> out rows 0:40, group 2p+1 -> rows 40:80
    w1s = np.zeros((128, 160), dtype=np.float32)
    for p in range(2):
        for sub in range(2):
            r0 = 32 * (2 * p + sub)
            c0 = 80 * p + 40 * sub
            w1s[r0:r0 + EMB, c0:c0 + HID] = W1[:, 0:EMB].T
            w1s[r0 + 16:r0 + 16 + EMB, c0:c0 + HID] = W1[:, EMB:2 * EMB].T
    b1e = np.concatenate([b1, b1]).reshape(80, 1).astype(np.float32)

    def wrap16(ids):
        # num_idxs wrapped across a core's 16 partitions: idx i lives at
        # [i % 16, i // 16]
        return ids.astype(np.int16).reshape(-1, 16).T.copy()

    shared = {"xtab": xtab, "w1s": w1s, "b1e": b1e, "tstk": tstk}
    in_maps = []
    for core in range(NCORES):
        xi = np.zeros((128, BC // 4 // 16), dtype=np.int16)
        for g in range(4):
            sl = slice(core * BC + g * 2048, core * BC + (g + 1) * 2048)
            xi[32 * g:32 * g + 16] = wrap16(worker_ids[sl])
            xi[32 * g + 16:32 * g + 32] = wrap16(project_ids[sl])
        m = dict(shared)
        m["xidx"] = xi
        in_maps.append(m)
    return in_maps


def _decode(results):
    # hstack col j holds batch row perm[j]: cols [0:4096] = h80 rows 0:40 =
    # pair0-A (batch 0:2048) ++ pair1-A (4096:6144); cols [4096:8192] =
    # rows 40:80 = pair0-B (2048:4096) ++ pair1-B (6144:8192)
    perm = np.concatenate([np.arange(0, 2048), np.arange(4096, 6144),
                           np.arange(2048, 4096), np.arange(6144, 8192)])
    idx = np.zeros((B,), dtype=np.int64)
    for core in range(NCORES):
        o = results[core]["out"]          # [128, 8*NTILES] uint32
        for t in range(NTILES):
            rows = core * BC + perm[t * 128:(t + 1) * 128]
            idx[rows] = o[:, 8 * t]
    return (idx + 1).astype(np.int32).reshape(B, 1)


def kernel(**inputs):
    from concourse.bass_utils import run_bass_kernel_spmd
    in_maps = _host_prep(inputs)
    if "nc1" not in _cache:
        _cache["nc1"] = _build(L=1)
    res = run_bass_kernel_spmd(_cache["nc1"], in_maps, core_ids=list(range(NCORES)))
    return _decode(res.results)

